# revision 19
# baseline (speedup 1.0000x reference)
"""Deformable multi-headed attention on 8 Trainium2 NeuronCores.

Sharding: (batch b, head-half) -> 8 cores. Core c handles batch c//2, heads
(c%2)*8 .. (c%2)*8+8. Per core: head-slice projections on PE; projected K/V
rows land in 16 per-m-tile DRAM chunk tables (with halos, so tile t's samples
always hit chunk t); one batched indirect DMA per tile gathers the bilinear
sample pairs; softmax attention over NK=5 samples runs on DVE; the output
projection runs on PE. Emission interleaves the projection (t+1) with
attention (t) so PE and DVE overlap. The host sums the two per-batch partials
and adds the output bias.

Self-contained: hardcodes B=4, M=2048, D=1024, H=16, NK=5.
"""
import sys, os
sys.path.insert(0, "/opt/trn_rl_repo")
import math
from contextlib import ExitStack

import numpy as np
import ml_dtypes

import concourse.bass as bass
import concourse.tile as tile
from concourse import mybir
from concourse import library_config
from concourse.bass import AP
from concourse.library_overlay import lower_extended_insts

bf16 = ml_dtypes.bfloat16
F32, BF, I32 = mybir.dt.float32, mybir.dt.bfloat16, mybir.dt.int32
I16 = mybir.dt.int16
ALU = mybir.AluOpType
ACTF = mybir.ActivationFunctionType
AXL = mybir.AxisListType

P = 128
B, M, D, H, NK = 4, 2048, 1024, 16, 5
HD = D // H            # 64
NH = 8                 # heads per core
DH = NH * HD           # 512
NT = M // P            # 16 m-tiles
L = float(M - 1)       # 2047.0
NDT = D // P           # 8 contraction tiles
NW = NH * NK           # 40
CW = 162               # chunk rows per head: 17 halo + 128 + 17 halo
G = 2 * NK             # 10 = (nk, pair) samples per head
SCL = float(np.float32(M) / np.float32(L))
ROWE = 2 * HD          # 128 elems per table row [K | V]


def emit_kernel(tc, ctx, io):
    nc = tc.nc
    wp = ctx.enter_context(tc.tile_pool(name="wp", bufs=1))
    cst = ctx.enter_context(tc.tile_pool(name="cst", bufs=1))
    xin = ctx.enter_context(tc.tile_pool(name="xin", bufs=3))
    kvp = ctx.enter_context(tc.tile_pool(name="kvp", bufs=2))
    res = ctx.enter_context(tc.tile_pool(name="res", bufs=NT))
    idxp = ctx.enter_context(tc.tile_pool(name="idxp", bufs=3))
    vgat = ctx.enter_context(tc.tile_pool(name="vgat", bufs=2))
    work = ctx.enter_context(tc.tile_pool(name="work", bufs=1))
    sfm = ctx.enter_context(tc.tile_pool(name="sfm", bufs=3))
    octx = ctx.enter_context(tc.tile_pool(name="octx", bufs=2))
    psp = ctx.enter_context(tc.tile_pool(name="psp", bufs=2, space="PSUM"))
    psf = ctx.enter_context(tc.tile_pool(name="psf", bufs=2, space="PSUM"))
    pst = ctx.enter_context(tc.tile_pool(name="pst", bufs=2, space="PSUM"))
    pso = ctx.enter_context(tc.tile_pool(name="pso", bufs=2, space="PSUM"))

    # per-m-tile chunk tables: row (h*CW + rel) holds [K[g], V[g]] where
    # g = c*128 - 17 + rel  (so tile c's sample rows rel = u - c*128 + 16,
    # u = j+1, always land inside chunk c; wrap zones use +/-2049).
    # One extra row so the 2-row overlapping gather view stays in bounds.
    tabc = [nc.dram_tensor(f"tabc{c}", [NH * CW + 1, ROWE], BF).ap() for c in range(NT)]
    jbufs = [nc.dram_tensor(f"jbuf{i}", [P, NW], I16).ap() for i in range(2)]

    # ---- resident weights (one batched DMA per array) ----
    def load_batched(name, rows, cols):
        nch = rows // P
        t = wp.tile([P, nch * cols], BF, tag=name)
        inap = AP(io[name].tensor, 0, [[cols, P], [P * cols, nch], [1, cols]])
        nc.sync.dma_start(t[:].rearrange("p (c e) -> p c e", c=nch), inap)
        return [t[:, c * cols:(c + 1) * cols] for c in range(nch)]

    wk = load_batched("wkT", D, DH)
    wv = load_batched("wvT", D, DH)
    wq = load_batched("wqT", D, DH)
    wo = load_batched("woT", DH, D)
    wch = load_batched("wcomb_hi", D, NW)
    wcl = load_batched("wcomb_lo", D, NW)

    # biases packed [1, 3*DH + 2*NW]: bk | bv | bq | bcomb_hi | bcomb_lo
    bt = wp.tile([1, 3 * DH + 2 * NW], BF, tag="biases")
    nc.sync.dma_start(bt[:], io["biases"][:])
    bias = {"bk": bt[:, 0:DH], "bv": bt[:, DH:2 * DH], "bq": bt[:, 2 * DH:3 * DH],
            "bcomb_hi": bt[:, 3 * DH:3 * DH + NW], "bcomb_lo": bt[:, 3 * DH + NW:3 * DH + 2 * NW]}

    ones1 = cst.tile([1, P], BF)
    nc.vector.memset(ones1[:], 1.0)
    identb = cst.tile([P, P], F32)
    from concourse.masks import make_identity
    make_identity(nc, identb[:])

    refI = cst.tile([P, NW], I32)
    i1 = nc.gpsimd.iota(refI[:], pattern=[[0, NH], [1, NK]], base=(-NK) // 2, channel_multiplier=0)
    refF = cst.tile([P, NW], F32)
    nc.vector.tensor_copy(refF[:], refI[:])
    # head chunk-base offsets (h*CW), f32 [P, 40]
    hoffI = cst.tile([P, NW], I32)
    i2 = nc.gpsimd.iota(hoffI[:], pattern=[[CW, NH], [0, NK]], base=0, channel_multiplier=0)
    hoffF = cst.tile([P, NW], F32)
    nc.vector.tensor_copy(hoffF[:], hoffI[:])
    locI = cst.tile([P, NT], I32)
    i3 = nc.gpsimd.iota(locI[:], pattern=[[P, NT]], base=0, channel_multiplier=1)
    locF = cst.tile([P, NT], F32)
    nc.vector.tensor_copy(locF[:], locI[:])

    # switch the Q7 extended-instruction library to mlp (for dma_gather) after
    # all standard-library gpsimd ops (the iotas) have retired.
    from concourse.tile_rust import add_dep_helper
    libload = nc.gpsimd.load_library(library_config.mlp)
    for ii in (i1, i2, i3):
        add_dep_helper(libload.ins, ii.ins, sync=True, reason="lib switch after iotas")

    # zero pads: chunk0 rel16 (= table row u=0, K[-1]=0); chunk15 rel145 (u=2049)
    zpad = cst.tile([P, ROWE], BF)
    nc.vector.memset(zpad[:], 0.0)
    for c, rel in ((0, 16), (NT - 1, 145)):
        padap = AP(tabc[c].tensor, rel * ROWE, [[CW * ROWE, NH], [1, ROWE]])
        nc.gpsimd.dma_start(padap, zpad[:NH, :])
    for c in range(NT):  # the bounds-padding row of each chunk
        nc.gpsimd.dma_start(AP(tabc[c].tensor, NH * CW * ROWE, [[1, ROWE]]), zpad[0:1, :])

    qh_res = [None] * NT
    j_res = [None] * NT
    w0_res = [None] * NT
    w1_res = [None] * NT

    # ---------------- phase A(t): projections, table writes, indices ----------------
    pso_res = [None] * NT

    def a_proj(t):
        ms = slice(t * P, (t + 1) * P)
        xqh = xin.tile([P, D], BF, tag="xqh")
        xql = xin.tile([P, D], BF, tag="xql")
        xk = xin.tile([P, D], BF, tag="xk")
        xv = xin.tile([P, D], BF, tag="xv")
        nc.sync.dma_start(xqh[:], io["xqt_hi"][ms, :])
        nc.sync.dma_start(xql[:], io["xqt_lo"][ms, :])
        nc.sync.dma_start(xk[:], io["xkt"][ms, :])
        nc.sync.dma_start(xv[:], io["xvt"][ms, :])

        def chunks(tl):
            return [tl[:, dt * P:(dt + 1) * P] for dt in range(NDT)]

        kT, vT, qT, qlT = chunks(xk), chunks(xv), chunks(xqh), chunks(xql)

        # K/V projections into one [P, 1024] tile laid out (h, [K|V], d)
        kvs = kvp.tile([P, 2 * DH], BF, tag="kvs")
        kvs4 = kvs[:].rearrange("p (h i d) -> p h i d", h=NH, i=2)
        for xT, w, bname, islot in ((kT, wk, "bk", 0), (vT, wv, "bv", 1)):
            ps = psp.tile([P, DH], F32, space="PSUM", tag="proj")
            for dt in range(NDT):
                nc.tensor.matmul(ps[:], lhsT=xT[dt], rhs=w[dt], start=(dt == 0), stop=False)
            nc.tensor.matmul(ps[:], lhsT=ones1[:], rhs=bias[bname], start=False, stop=True)
            nc.scalar.copy(kvs4[:, :, islot, :], ps[:].rearrange("p (h d) -> p h d", h=NH))

        # table writes: row u = m+1 -> chunk t rel 17..144; halos to chunks t+-1;
        # wrap specials for t=0 (tail->chunk15) and t=15 (head->chunk0)
        inap = kvs[:].rearrange("p (h x) -> p h x", h=NH)
        def w_chunk(c, rel0, p0, np_):
            out = AP(tabc[c].tensor, rel0 * ROWE, [[ROWE, np_], [CW * ROWE, NH], [1, ROWE]])
            src = kvs[p0:p0 + np_].rearrange("p (h x) -> p h x", h=NH)
            nc.sync.dma_start(out, src)
        w_chunk(t, 17, 0, P)                      # main: u=m+1, rel=u-t*128+16
        if t < NT - 1:
            w_chunk(t + 1, 0, P - 17, 17)         # head halo: rel 0..16
        else:
            w_chunk(0, 0, P - 16, 16)             # wrap: K[2032..2047] at rel 0..15
        if t > 0:
            w_chunk(t - 1, 145, 0, 17)            # tail halo: rel 145..161
        else:
            w_chunk(NT - 1, 146, 0, 16)           # wrap: K[0..15] at rel 146..161

        # Q projection
        psq = psp.tile([P, DH], F32, space="PSUM", tag="proj")
        for dt in range(NDT):
            nc.tensor.matmul(psq[:], lhsT=qT[dt], rhs=wq[dt], start=(dt == 0), stop=False)
        nc.tensor.matmul(psq[:], lhsT=ones1[:], rhs=bias["bq"], start=False, stop=True)
        Qs = res.tile([P, DH], BF, tag="Qs")
        nc.scalar.copy(Qs[:], psq[:])
        qh_res[t] = Qs

        # offsets: off = q @ Wcomb + bcomb (hi/lo bf16 decomposition, f32 psum)
        pso_t = psf.tile([P, NW], F32, space="PSUM", tag="off")
        first = True
        for xT, w in ((qT, wch), (qlT, wch), (qT, wcl)):
            for dt in range(NDT):
                nc.tensor.matmul(pso_t[:], lhsT=xT[dt], rhs=w[dt], start=first, stop=False)
                first = False
        nc.tensor.matmul(pso_t[:], lhsT=ones1[:], rhs=bias["bcomb_hi"], start=False, stop=False)
        nc.tensor.matmul(pso_t[:], lhsT=ones1[:], rhs=bias["bcomb_lo"], start=False, stop=True)
        pso_res[t] = pso_t

    def a_idx(t):
        pso_t = pso_res[t]
        # index math (f32, [P, 40])
        t1 = idxp.tile([P, NW], F32, tag="t1")
        nc.vector.tensor_tensor(out=t1[:], in0=pso_t[:], in1=refF[:], op=ALU.add)
        r = idxp.tile([P, NW], F32, tag="r")
        nc.vector.tensor_scalar(out=r[:], in0=t1[:], scalar1=locF[:, t:t + 1], scalar2=None, op0=ALU.add)
        m1 = idxp.tile([P, NW], F32, tag="m1")
        nc.vector.tensor_scalar(out=m1[:], in0=r[:], scalar1=L, scalar2=None, op0=ALU.is_ge)
        r2 = idxp.tile([P, NW], F32, tag="r2")
        nc.vector.scalar_tensor_tensor(out=r2[:], in0=m1[:], scalar=-L, in1=r[:], op0=ALU.mult, op1=ALU.add)
        m2 = idxp.tile([P, NW], F32, tag="m2")
        nc.vector.tensor_scalar(out=m2[:], in0=r2[:], scalar1=0.0, scalar2=None, op0=ALU.is_lt)
        lx = idxp.tile([P, NW], F32, tag="lx")
        nc.vector.scalar_tensor_tensor(out=lx[:], in0=m2[:], scalar=L, in1=r2[:], op0=ALU.mult, op1=ALU.add)
        # jf = ix + 1 = lx*(M/L) + 0.5 ; u0 = floor(jf)
        jf = idxp.tile([P, NW], F32, tag="jf")
        nc.vector.tensor_scalar(out=jf[:], in0=lx[:], scalar1=SCL, scalar2=0.5, op0=ALU.mult, op1=ALU.add)
        jraw = idxp.tile([P, NW], I32, tag="jraw")
        nc.vector.tensor_copy(jraw[:], jf[:])          # cast (rounding mode may vary)
        jtr = idxp.tile([P, NW], F32, tag="jtr")
        nc.vector.tensor_copy(jtr[:], jraw[:])
        cgt = idxp.tile([P, NW], F32, tag="cgt")
        nc.vector.tensor_tensor(out=cgt[:], in0=jtr[:], in1=jf[:], op=ALU.is_gt)
        jtr2 = idxp.tile([P, NW], F32, tag="jtr2")
        nc.vector.tensor_tensor(out=jtr2[:], in0=jtr[:], in1=cgt[:], op=ALU.subtract)
        dx = idxp.tile([P, NW], F32, tag="dx")
        nc.vector.tensor_tensor(out=dx[:], in0=jf[:], in1=jtr2[:], op=ALU.subtract)  # frac(ix)
        # chunk-relative row: rel = u0 + 16 - t*128 + (m1 - m2)*2049
        c1 = idxp.tile([P, NW], F32, tag="c1")
        nc.vector.tensor_tensor(out=c1[:], in0=m1[:], in1=m2[:], op=ALU.subtract)
        jw = idxp.tile([P, NW], F32, tag="jw")
        nc.vector.scalar_tensor_tensor(out=jw[:], in0=c1[:], scalar=2049.0, in1=jtr2[:],
                                       op0=ALU.mult, op1=ALU.add)
        jw2 = idxp.tile([P, NW], F32, tag="jw2")
        nc.vector.tensor_scalar(out=jw2[:], in0=jw[:], scalar1=float(16 - t * P), scalar2=None, op0=ALU.add)
        jw3 = idxp.tile([P, NW], F32, tag="jw3")
        nc.vector.tensor_tensor(out=jw3[:], in0=jw2[:], in1=hoffF[:], op=ALU.add)  # + h*CW
        # w0 = (1-dx)/16, w1 = dx/16  (0.5 bilinear * 1/8 score scale)
        w0t = res.tile([P, NW], F32, tag="w0")
        nc.vector.tensor_scalar(out=w0t[:], in0=dx[:], scalar1=-1.0 / 16.0, scalar2=1.0 / 16.0,
                                op0=ALU.mult, op1=ALU.add)
        w1t = res.tile([P, NW], F32, tag="w1")
        nc.vector.tensor_scalar(out=w1t[:], in0=dx[:], scalar1=1.0 / 16.0, scalar2=None, op0=ALU.mult)
        jfin = res.tile([P, NW], I16, tag="jfin")
        nc.vector.tensor_copy(jfin[:], jw3[:])         # exact (integral, < 1296)
        j_res[t] = jfin
        w0_res[t] = w0t
        w1_res[t] = w1t

    # ---------------- phase B(t): gather, attention, output ----------------
    def phase_b(t):
        # batched gather via dma_gather: index i = c*128 + p fetches 256 elems
        # (2 chunk rows = [K[j] V[j] K[j+1] V[j+1]]) -> Kvg[p, c*256:(c+1)*256].
        # HW ucode reads idx16[16 + i%16, i//16]; fill partitions 16:32 with the
        # folded layout (and everything else with per-partition values so the
        # interpreter's bounds checks stay happy).
        ji = j_res[t]
        jb = jbufs[t % 2]
        nc.sync.dma_start(jb[:], ji[:])
        idx16 = idxp.tile([P, NW * 8], I16, tag="idx16")
        nc.vector.tensor_copy(idx16[:].rearrange("p (c q) -> p c q", c=NW),
                              ji[:].rearrange("p (c o) -> p c o", o=1).to_broadcast([P, NW, 8]))
        dst = idx16[16:32, :].rearrange("p (c q) -> p c q", c=NW)
        src = AP(jb.tensor, 0, [[NW, 16], [1, NW], [16 * NW, 8]])
        nc.sync.dma_start(dst, src)
        Kvg = vgat.tile([P, NW * 2 * ROWE], BF, tag="Kvg")
        nc.gpsimd.dma_gather(
            out_ap=Kvg[:].rearrange("p (c e) -> p c e", c=NW),
            in_ap=AP(tabc[t].tensor, 0, [[ROWE, NH * CW], [1, 2 * ROWE]]),
            idxs_ap=idx16[:],
            num_idxs=P * NW, num_idxs_reg=P * NW,
            elem_size=2 * ROWE, elem_step=ROWE,
            single_packet=False,
        )

        kv = Kvg[:]
        kviewK = AP(kv.tensor, kv.offset, [kv.ap[0], [NK * 4 * HD, NH], [2 * HD, G], [1, HD]])
        kviewV = AP(kv.tensor, kv.offset + HD, [kv.ap[0], [NK * 4 * HD, NH], [2 * HD, G], [1, HD]])
        q_b = qh_res[t][:].rearrange("p (h o d) -> p h o d", h=NH, o=1).to_broadcast([P, NH, G, HD])

        # ---- K side: prod + tree reduce -> d01 [p, (h,nk,i)=80] ----
        prodK = work.tile([P, NH * G * HD], BF, tag="prod")
        pk = prodK[:].rearrange("p (h g d) -> p h g d", h=NH, g=G)
        nc.vector.tensor_tensor(out=pk, in0=kviewK, in1=q_b, op=ALU.mult)
        ka1 = work.tile([P, NH * G * 32], BF, tag="ka1")
        nc.vector.tensor_tensor(out=ka1[:].rearrange("p (h g d) -> p h g d", h=NH, g=G),
                                in0=pk[:, :, :, 0:32], in1=pk[:, :, :, 32:64], op=ALU.add)
        ka14 = ka1[:].rearrange("p (h g d) -> p h g d", h=NH, g=G)
        ka2 = work.tile([P, NH * G * 16], BF, tag="ka2")
        nc.vector.tensor_tensor(out=ka2[:].rearrange("p (h g d) -> p h g d", h=NH, g=G),
                                in0=ka14[:, :, :, 0:16], in1=ka14[:, :, :, 16:32], op=ALU.add)
        d01 = sfm.tile([P, NH * G], F32, tag="d01")
        nc.vector.tensor_reduce(d01[:], ka2[:].rearrange("p (a d) -> p a d", d=16),
                                axis=AXL.X, op=ALU.add)

        # ---- softmax over nk (no max-sub; scores are O(1)) ----
        d014 = d01[:].rearrange("p (a i) -> p a i", i=2)
        sc = sfm.tile([P, NW], F32, tag="sc")
        nc.vector.tensor_tensor(out=sc[:].rearrange("p (a o) -> p a o", o=1),
                                in0=d014[:, :, 0:1], in1=w0_res[t][:].rearrange("p (a o) -> p a o", o=1),
                                op=ALU.mult)
        sc1 = sfm.tile([P, NW], F32, tag="sc1")
        nc.vector.tensor_tensor(out=sc1[:].rearrange("p (a o) -> p a o", o=1),
                                in0=d014[:, :, 1:2], in1=w1_res[t][:].rearrange("p (a o) -> p a o", o=1),
                                op=ALU.mult)
        nc.vector.tensor_tensor(out=sc[:], in0=sc[:], in1=sc1[:], op=ALU.add)
        ex = sfm.tile([P, NW], F32, tag="ex")
        nc.scalar.activation(ex[:], sc[:], ACTF.Exp)
        sm = sfm.tile([P, NH], F32, tag="sm")
        nc.vector.tensor_reduce(sm[:], ex[:].rearrange("p (h k) -> p h k", k=NK),
                                axis=AXL.X, op=ALU.add)
        rec = sfm.tile([P, NH], F32, tag="rec")
        nc.vector.reciprocal(rec[:], sm[:])
        at = sfm.tile([P, NW], F32, tag="at")
        rec_b = rec[:].rearrange("p (h o) -> p h o", o=1).to_broadcast([P, NH, NK])
        nc.vector.tensor_tensor(out=at[:].rearrange("p (h k) -> p h k", k=NK),
                                in0=ex[:].rearrange("p (h k) -> p h k", k=NK),
                                in1=rec_b, op=ALU.mult)
        # alpha (bf16) interleaved [p, (h, nk, i)=80]: at * w * 8
        alI = sfm.tile([P, NH * G], BF, tag="alI")
        alI4 = alI[:].rearrange("p (a i) -> p a i", i=2)
        nc.vector.scalar_tensor_tensor(out=alI4[:, :, 0:1], in0=at[:].rearrange("p (a o) -> p a o", o=1),
                                       scalar=8.0, in1=w0_res[t][:].rearrange("p (a o) -> p a o", o=1),
                                       op0=ALU.mult, op1=ALU.mult)
        nc.vector.scalar_tensor_tensor(out=alI4[:, :, 1:2], in0=at[:].rearrange("p (a o) -> p a o", o=1),
                                       scalar=8.0, in1=w1_res[t][:].rearrange("p (a o) -> p a o", o=1),
                                       op0=ALU.mult, op1=ALU.mult)
        # ---- V side: prod + i-fold + nk tree -> ctx [p, (h d)=512] f32 ----
        # heads 4-7: direct 1x multiply (broadcast alpha) while ACT expands
        # alpha for heads 0-3; those then multiply at 2x.
        NHH = NH // 2
        alx = work.tile([P, NHH * G * HD], BF, tag="alx")
        nc.scalar.copy(alx[:].rearrange("p (h g d) -> p h g d", h=NHH, g=G),
                       alI[:, :NHH * G].rearrange("p (h g o) -> p h g o", h=NHH, g=G)
                       .to_broadcast([P, NHH, G, HD]))
        al_hi = alI[:, NHH * G:].rearrange("p (h g o) -> p h g o", h=NHH, g=G).to_broadcast([P, NHH, G, HD])
        prodV = work.tile([P, NH * G * HD], BF, tag="prod")
        pv = prodV[:].rearrange("p (h g d) -> p h g d", h=NH, g=G)
        kvV_hi = AP(kv.tensor, kv.offset + HD + NHH * NK * 4 * HD,
                    [kv.ap[0], [NK * 4 * HD, NHH], [2 * HD, G], [1, HD]])
        nc.vector.tensor_tensor(out=pv[:, NHH:, :, :], in0=kvV_hi, in1=al_hi, op=ALU.mult)
        kvV_lo = AP(kv.tensor, kv.offset + HD,
                    [kv.ap[0], [NK * 4 * HD, NHH], [2 * HD, G], [1, HD]])
        nc.vector.tensor_tensor(out=pv[:, :NHH, :, :], in0=kvV_lo,
                                in1=alx[:].rearrange("p (h g d) -> p h g d", h=NHH, g=G), op=ALU.mult)
        pvi = prodV[:].rearrange("p (h nk i d) -> p h nk i d", h=NH, nk=NK, i=2)
        v1 = work.tile([P, NH * NK * HD], BF, tag="v1")
        v14 = v1[:].rearrange("p (h k d) -> p h k d", h=NH, k=NK)
        nc.vector.tensor_tensor(out=v14, in0=pvi[:, :, :, 0, :], in1=pvi[:, :, :, 1, :], op=ALU.add)
        vt1 = work.tile([P, NH * 2 * HD], BF, tag="vt1")
        vt14 = vt1[:].rearrange("p (h k d) -> p h k d", h=NH, k=2)
        nc.vector.tensor_tensor(out=vt14, in0=v14[:, :, 0:2, :], in1=v14[:, :, 2:4, :], op=ALU.add)
        vt2 = work.tile([P, NH * HD], BF, tag="vt2")
        vt24 = vt2[:].rearrange("p (h o d) -> p h o d", h=NH, o=1)
        nc.vector.tensor_tensor(out=vt24, in0=vt14[:, :, 0:1, :], in1=vt14[:, :, 1:2, :], op=ALU.add)
        ctx_t = octx.tile([P, DH], F32, tag="ctx")
        nc.vector.tensor_tensor(out=ctx_t[:].rearrange("p (h o d) -> p h o d", h=NH, o=1),
                                in0=vt24, in1=v14[:, :, 4:5, :], op=ALU.add)

        # ---- output projection: transpose ctx on PE, then ctx @ woT ----
        cT = octx.tile([P, 4 * P], BF, tag="cT")
        for c in range(4):
            pt = pst.tile([P, P], F32, space="PSUM", tag="pt")
            nc.tensor.transpose(pt[:], ctx_t[:, c * P:(c + 1) * P], identb[:])
            nc.scalar.copy(cT[:, c * P:(c + 1) * P], pt[:])
        osb = octx.tile([P, D], BF, tag="osb")
        for half in range(2):
            ps = pso.tile([P, 512], F32, space="PSUM", tag="out")
            for c in range(4):
                nc.tensor.matmul(ps[:], lhsT=cT[:, c * P:(c + 1) * P],
                                 rhs=wo[c][:, half * 512:(half + 1) * 512],
                                 start=(c == 0), stop=(c == 3))
            nc.scalar.copy(osb[:, half * 512:(half + 1) * 512], ps[:])
        nc.sync.dma_start(io["outp"][t * P:(t + 1) * P, :], osb[:])

    # staggered emission: projections lead, index math trails, attention last
    a_proj(NT - 1)
    a_proj(0)
    a_proj(1)
    a_idx(NT - 1)
    a_idx(0)
    for t in range(NT - 1):
        if t + 2 <= NT - 2:
            a_proj(t + 2)
        if t + 1 <= NT - 2:
            a_idx(t + 1)
        phase_b(t)
    phase_b(NT - 1)


def build_program(split_waits=True):
    nc = bass.Bass("TRN2", target_bir_lowering=False, debug=False)
    io = {}
    def inp(name, shape, dt):
        io[name] = nc.dram_tensor(name, shape, dt, kind="ExternalInput").ap()
    inp("xqt_hi", [M, D], BF); inp("xqt_lo", [M, D], BF)
    inp("xkt", [M, D], BF); inp("xvt", [M, D], BF)
    inp("wkT", [D, DH], BF); inp("wvT", [D, DH], BF); inp("wqT", [D, DH], BF)
    inp("woT", [DH, D], BF)
    inp("wcomb_hi", [D, NW], BF); inp("wcomb_lo", [D, NW], BF)
    inp("biases", [1, 3 * DH + 2 * NW], BF)
    io["outp"] = nc.dram_tensor("outp", [M, D], BF, kind="ExternalOutput").ap()
    with tile.TileContext(nc) as tc:
        with ExitStack() as ctx:
            emit_kernel(tc, ctx, io)
    lower_extended_insts(nc)
    if split_waits:
        _split_dma_waits(nc)
    return nc


def _split_dma_waits(nc, max_waits=1):
    """Walrus instruction encodings accept a limited number of sync waits.
    Move excess waits onto preceding same-engine NoOps (engine dispatch is
    in-order, so waiting on the NoOp before the instruction is equivalent)."""
    for fn in nc.m.functions:
        for bb in fn.blocks:
            insts = bb.instructions
            out = []
            changed = False
            for inst in insts:
                si = getattr(inst, "sync_info", None)
                if (si is not None and si.on_wait and len(si.on_wait) > max_waits
                        and type(inst).__name__ != "InstCall"):
                    waits = list(si.on_wait)
                    keep = waits[-max_waits:]
                    move = waits[:-max_waits]
                    k = 0
                    while move:
                        chunk, move = move[:max_waits], move[max_waits:]
                        out.append(mybir.InstNoOp(
                            name=f"{inst.name}-wsplit{k}", engine=inst.engine,
                            ins=[], outs=[],
                            sync_info=mybir.SyncInfo(on_wait=chunk, on_update=[])))
                        k += 1
                    inst.sync_info = mybir.SyncInfo(on_wait=keep, on_update=si.on_update)
                    changed = True
                out.append(inst)
            if changed:
                bb.instructions = out


_NC_CACHE = None

def get_program():
    global _NC_CACHE
    if _NC_CACHE is None:
        _NC_CACHE = build_program()
    return _NC_CACHE


def _tiled_T(a_f32, want_lo=True):
    """[M, D] f32 -> transposed bf16 in per-tile layout: slab row (t*128+p)
    holds aT[c*128+p, t*128:(t+1)*128] for c=0..7 (2KB/partition descriptors)."""
    aT = np.ascontiguousarray(a_f32.T)                      # [D, M]
    hi = aT.astype(bf16)
    lo = (aT - hi.astype(np.float32)).astype(bf16) if want_lo else None
    def lay(x):
        return np.ascontiguousarray(
            x.reshape(NDT, P, NT, P).transpose(2, 1, 0, 3).reshape(M, D))
    return lay(hi), (lay(lo) if want_lo else None)


def make_in_maps(inputs):
    """Build the 8 per-core input dicts from full inputs."""
    q = np.asarray(inputs["q"], np.float32)
    k = np.asarray(inputs["k"], np.float32)
    v = np.asarray(inputs["v"], np.float32)
    Wk = np.asarray(inputs["Wk"], np.float32); bk = np.asarray(inputs["bk"], np.float32)
    Wv = np.asarray(inputs["Wv"], np.float32); bv = np.asarray(inputs["bv"], np.float32)
    Wq = np.asarray(inputs["Wq"], np.float32); bq = np.asarray(inputs["bq"], np.float32)
    Woff = np.asarray(inputs["Woff"], np.float32); boff = np.asarray(inputs["boff"], np.float32)
    Wo = np.asarray(inputs["Wo"], np.float32)

    # fold offset projection: off = Qp @ Woff.T + boff = q @ Wcomb + bcomb
    Wcomb = (Wq.astype(np.float64).T @ Woff.astype(np.float64).T).astype(np.float32)
    bcomb = (bq.astype(np.float64) @ Woff.astype(np.float64).T + boff).astype(np.float32)
    Wcomb_hi = Wcomb.astype(bf16)
    Wcomb_lo = (Wcomb - Wcomb_hi.astype(np.float32)).astype(bf16)
    bcomb_hi = bcomb.astype(bf16)
    bcomb_lo = (bcomb - bcomb_hi.astype(np.float32)).astype(bf16)

    per_b = []
    for b in range(B):
        qhi, qlo = _tiled_T(q[b])
        khi, _ = _tiled_T(k[b], want_lo=False)
        vhi, _ = _tiled_T(v[b], want_lo=False)
        per_b.append((qhi, qlo, khi, vhi))

    in_maps = []
    for c in range(8):
        b, half = c // 2, c % 2
        cols = slice(half * DH, (half + 1) * DH)
        jcols = slice(half * NW, (half + 1) * NW)
        qhi, qlo, khi, vhi = per_b[b]
        biases = np.concatenate([
            bk[cols].astype(bf16), bv[cols].astype(bf16), bq[cols].astype(bf16),
            bcomb_hi[jcols], bcomb_lo[jcols]])[None, :]
        m = {
            "xqt_hi": qhi, "xqt_lo": qlo, "xkt": khi, "xvt": vhi,
            "wkT": np.ascontiguousarray(Wk.T[:, cols]).astype(bf16),
            "wvT": np.ascontiguousarray(Wv.T[:, cols]).astype(bf16),
            "wqT": np.ascontiguousarray(Wq.T[:, cols]).astype(bf16),
            "woT": np.ascontiguousarray(Wo[:, cols].T).astype(bf16),
            "wcomb_hi": np.ascontiguousarray(Wcomb_hi[:, jcols]),
            "wcomb_lo": np.ascontiguousarray(Wcomb_lo[:, jcols]),
            "biases": np.ascontiguousarray(biases),
        }
        in_maps.append(m)
    return in_maps


def kernel(**inputs):
    from concourse.bass_utils import run_bass_kernel_spmd
    nc = get_program()
    in_maps = make_in_maps(inputs)
    res = run_bass_kernel_spmd(nc, in_maps, list(range(8)))
    bo = np.asarray(inputs["bo"], np.float32)
    out = np.empty((B, M, D), np.float32)
    for b in range(B):
        out[b] = (res.results[2 * b]["outp"].astype(np.float32)
                  + res.results[2 * b + 1]["outp"].astype(np.float32) + bo)
    return out
